# revision 45
# baseline (speedup 1.0000x reference)
"""Trainium2 Bass kernel for nn_BoundaryDetectionLoss.

Computes, for start/end (probs, targets) pairs of shape (64, 131072):
    w   = 1 + exp(-dist_to_nearest_boundary / 5)     (distance transform)
    bce = (1-z)*x + (1+z)*softplus(-x)               (pos_weight = 2)
    loss = mean(bce * w)   per pair; total = (start_loss + end_loss)/2

Identity used on device (per pair, z in {0,1}, e = exp(-dist/5), zt = 2z-1):
    sum(bce*w) = sum(x*e) + sum(sp*e) - sum(x*zt) + sum(sp*zt) + 2*sum(sp)
where sp = softplus(-x).  Proof: expanding (1+e)((1-z)x + (1+z)sp) with
z*e = z (e == 1 exactly at boundaries) gives
    sum(x+sp) + sum((x+sp)e) + 2 sum((sp-x)z);
-sum(x*zt) = sum(x) - 2 sum(x*z) folds the plain x sum into the zt dot, and
+sum(sp*zt) = 2 sum(sp*z) - sum(sp) leaves a 2*sum(sp) residue, recovered
for free from the softplus pass's accum_out.

Device algorithm (per core, data-parallel over 8 rows of B=64):
  - Host sends zt = 2*z - 1 (padded with -1) and x, both pre-cast to fp16
    (halves HBM traffic; x rounding errors are random-sign and average out
    ~1e-6 in the mean, z/zt are exact in fp16). Plain HWDGE DMAs.
  - e[t] = exp(-dist[t]/5) as a decayed-max field via two DVE
    tensor_tensor_scan passes (op0=mult by a=exp(-1/5), op1=max) over
    halo'd sub-windows; scan state is fp32 internally.  max(a*state, -1)
    never binds because state >= 0, so the zt-valued scan input produces
    the same field as z would.
  - sp = softplus(-x) = ln(1 + exp(-x)) via two ACT passes per x quarter
    (Exp then Ln with bias=1, both in the natural_log_exp_and_others table
    set -- one table load; Softplus itself is absent from production PWP).
  - PE: per 128-block, psum_e += e_blk^T @ [x|sp], psum_z += zt_blk^T @ [x|sp];
    host sums the PSUM diagonals and combines with the identity above.
"""

import sys

for _p in ("/opt/trn_rl_repo", "/root/.axon_site/_ro/trn_rl_repo"):
    if _p not in sys.path:
        sys.path.append(_p)

import numpy as np

# ---------------------------------------------------------------- config
B_FULL = 64
T_FULL = 131072
N_CORES = 8
ROWS = B_FULL // N_CORES  # 8 rows per core
DECAY = float(np.exp(np.float32(-0.2), dtype=np.float32))  # a = exp(-1/5)


class Cfg:
    def __init__(self, rows=8, chunks=16, scan_splits=(), n_x=4, halo=48):
        self.rows = rows
        self.chunks = chunks
        self.halo = halo
        self.chunk_len = T_FULL // chunks  # 8192
        self.T = T_FULL
        self.parts = rows * chunks
        assert self.parts <= 128
        # scan sub-tiles per chunk: boundaries at scan_splits (asymmetric ok)
        bounds = [0, *scan_splits, self.chunk_len]
        self.subs = [
            (bounds[i], bounds[i + 1] - bounds[i]) for i in range(len(bounds) - 1)
        ]
        self.n_scan = len(self.subs)
        self.max_W = max(ln for _, ln in self.subs) + 2 * halo
        self.n_x = n_x        # x quarters per chunk (ACT granularity)
        self.XL = self.chunk_len // n_x
        self.blk = 128
        self.n_blk = self.chunk_len // 128  # 64
        self.x_ring = "sp"   # which DMA ring carries x: act | sp | gpsimd


PROD_CFG = Cfg()
PAIRS = (("start_probs", "start_targets"), ("end_probs", "end_targets"))


def _build_body(nc, tc, cfg, dram_in, psums, const_a, acc, zpool, xpool,
                spool, tpool, bass, mybir, ablate=frozenset()):
    f32 = mybir.dt.float32
    bf16 = mybir.dt.float16
    AF = mybir.ActivationFunctionType
    OP = mybir.AluOpType
    P, H, CL = cfg.parts, cfg.halo, cfg.chunk_len
    Tp = cfg.T + 2 * H  # host-padded row length

    # Per-pair persistent tiles
    zt = {}   # zt (2z-1) with halo window, bf16
    xs = {}   # [x | sp] stacked moving operand, bf16
    es = {}   # per scan sub-tile: final e field (bwd output), bf16

    def dma_z_seg(pi, i):
        # Disjoint dest segments (seg 0 = first scan window, the rest up to
        # the next window end) so sub-scans start as their data lands
        # without overlapping DMA writes.  SP HWDGE ring.
        px, pz = PAIRS[pi]
        zd = dram_in[pz]
        if pi not in zt:
            zt[pi] = zpool.tile([P, CL + 2 * H], bf16, tag="zt", name=f"zt{pi}")
        ends = [st + ln + 2 * H for st, ln in cfg.subs]
        bounds = [0] + ends
        lo, hi = bounds[i], bounds[i + 1]
        win = bass.AP(
            zd,
            lo,
            [[Tp, cfg.rows], [CL, cfg.chunks], [1, hi - lo]],
        )
        nc.sync.dma_start(zt[pi][:, lo:hi], win)

    def dma_x(pi, h):
        # x halves; SP ring measured fastest (ACT-ring DMAs contend with
        # ACT compute dispatch: 112.6us vs 99.0us; gpsimd/SWDGE: 112.0us).
        px, pz = PAIRS[pi]
        xd = dram_in[px]
        HL = CL // 2
        if pi not in xs:
            xs[pi] = xpool.tile([P, 2 * CL], bf16, tag="xs", name=f"xs{pi}")
        win = bass.AP(
            xd,
            h * HL,
            [[cfg.T, cfg.rows], [CL, cfg.chunks], [1, HL]],
        )
        eng = {"act": nc.scalar, "sp": nc.sync, "gpsimd": nc.gpsimd}[cfg.x_ring]
        eng.dma_start(xs[pi][:, h * HL : (h + 1) * HL], win)

    def scan_fwd(pi, s):
        # ef_s = fwd decayed-max over window [start, start + len + 2H)
        st, ln = cfg.subs[s]
        W = ln + 2 * H
        ef = spool.tile([P, W], bf16, tag="ef", name=f"ef{pi}_{s}")
        zw = zt[pi][:, st : st + W]
        # 1-elem same-engine op absorbs the extra sync waits (STT-class ops
        # have one ISA wait slot; see _split_multiwaits)
        nc.vector.tensor_tensor(ef[:, 0:1], zw[:, 0:1], const_a[:, 0:1], OP.max)
        nc.vector.tensor_tensor_scan(ef[:], const_a[:, :W], zw, 0.0,
                                     OP.mult, OP.max)
        return ef

    def scan_bwd(pi, s, ef):
        st, ln = cfg.subs[s]
        W = ln + 2 * H
        e = spool.tile([P, W], bf16, tag="es", name=f"es{pi}_{s}")
        nc.vector.tensor_tensor(e[:, 0:1], ef[:, 0:1], const_a[:, 0:1], OP.max)
        nc.vector.tensor_tensor_scan(e[:, ::-1], const_a[:, W - 1::-1],
                                     ef[:, ::-1], 0.0, OP.mult, OP.max)
        es[(pi, s)] = e

    def act_sp(pi, q):
        sl = slice(q * cfg.XL, (q + 1) * cfg.XL)
        c = pi * cfg.n_x + q
        texp = tpool.tile([P, cfg.XL], f32, tag="texp", name=f"texp{pi}_{q}")
        nc.scalar.activation(texp[:], xs[pi][:, sl], AF.Exp, scale=-1.0)
        nc.scalar.activation(
            xs[pi][:, CL + q * cfg.XL : CL + (q + 1) * cfg.XL],
            texp[:], AF.Ln, bias=1.0,
            accum_out=acc[:, c : c + 1],
        )

    def pe_blocks(pi, blo, bhi):
        # z-dots first (gated only on zt DMA + sp), then e-dots (gated on
        # the bwd scan, which finishes later) — keeps PE fed while the last
        # scan is still running.
        xs3 = xs[pi][:].rearrange("p (g f) -> p g f", g=2)
        for b in range(blo, bhi):
            z_blk = zt[pi][:, H + b * cfg.blk : H + (b + 1) * cfg.blk]
            rhs = xs3[:, :, b * cfg.blk : (b + 1) * cfg.blk]
            nc.tensor.matmul(psums[2 * pi + 1][:], z_blk, rhs,
                             start=b == 0, stop=b == cfg.n_blk - 1)
        for b in range(blo, bhi):
            pos = b * cfg.blk
            s = max(i for i, (st, ln) in enumerate(cfg.subs) if st <= pos)
            if (pi, s) not in es:
                continue  # scan ablated
            st, ln = cfg.subs[s]
            off = pos - st + H  # offset within es window
            e_blk = es[(pi, s)][:, off : off + cfg.blk]
            rhs = xs3[:, :, pos : pos + cfg.blk]
            nc.tensor.matmul(psums[2 * pi][:], e_blk, rhs,
                             start=b == 0, stop=b == cfg.n_blk - 1)

    # ---- issue order: pipelined across pairs
    # DMA ring (single HWDGE/SP queue, FIFO): first z seg for the scan chain
    # (DVE is the longest engine), then the first x quarter for ACT, then
    # the rest.
    dma_z_seg(0, 0)
    dma_x(0, 0)
    for i in range(1, cfg.n_scan):
        dma_z_seg(0, i)
    dma_x(0, 1)
    for i in range(cfg.n_scan):
        dma_z_seg(1, i)
    for h in range(2):
        dma_x(1, h)

    if "noact" not in ablate:
        for pi in range(2):
            for q in range(cfg.n_x):
                act_sp(pi, q)
    else:
        # sp half of xs must still be written once for the PE reads
        for pi in range(2):
            nc.vector.memset(xs[pi][:, CL : 2 * CL], 0.5)

    if "noscan" not in ablate:
        for pi in range(2):
            for s in range(cfg.n_scan):
                ef = scan_fwd(pi, s)
                scan_bwd(pi, s, ef)

    if "nope" not in ablate:
        # PE per scan sub-tile granularity
        for pi in range(2):
            bper = cfg.n_blk // cfg.n_scan
            for s in range(cfg.n_scan):
                pe_blocks(pi, s * bper, (s + 1) * bper)


def build_nc(cfg: Cfg, split_waits=True, loop_n=1, ablate=frozenset()):
    """Build the per-core Bass program. Returns nc."""
    import concourse.bass as bass
    import concourse.tile as tile
    import concourse.mybir as mybir

    f32 = mybir.dt.float32

    nc = bass.Bass()
    f16 = mybir.dt.float16
    dram_in = {}
    for px, pz in PAIRS:
        dram_in[px] = nc.dram_tensor(px, [cfg.rows, cfg.T], f16, kind="ExternalInput")
        # zt arrives host-transformed (2z-1) and host-padded with -1, halo H
        dram_in[pz] = nc.dram_tensor(
            pz, [cfg.rows, cfg.T + 2 * cfg.halo], f16, kind="ExternalInput"
        )
    # dots: [pair*2+{e,z}, blk, 2*blk]
    dots_out = nc.dram_tensor(
        "dots", [4, cfg.blk, 2 * cfg.blk], f32, kind="ExternalOutput"
    )
    n_acc = 2 * cfg.n_x
    acc_out = nc.dram_tensor("acc", [cfg.parts, n_acc], f32, kind="ExternalOutput")

    with tile.TileContext(nc) as tc:
        with (
            tc.tile_pool(name="const", bufs=1) as cpool,
            tc.tile_pool(name="zt", bufs=2) as zpool,
            tc.tile_pool(name="xs", bufs=2) as xpool,
            tc.tile_pool(name="scan", bufs=2 * cfg.n_scan) as spool,
            tc.tile_pool(name="texp", bufs=2) as tpool,
            tc.tile_pool(name="psum", bufs=1, space="PSUM") as ppool,
            tc.tile_pool(name="outp", bufs=1) as opool,
        ):
            bf16 = mybir.dt.float16
            const_a = cpool.tile([cfg.parts, cfg.max_W], bf16, tag="ca")
            nc.vector.memset(const_a[:], DECAY)
            acc = cpool.tile([cfg.parts, 2 * cfg.n_x], f32, tag="acc")

            psums = [
                ppool.tile([cfg.blk, 2 * cfg.blk], f32, tag=f"ps{i}", name=f"ps{i}")
                for i in range(4)
            ]

            import contextlib

            loop_cm = (
                tc.For_i(0, loop_n, 1, hint_engines=(mybir.EngineType.PE,))
                if loop_n > 1
                else contextlib.nullcontext()
            )
            with loop_cm:
                _build_body(nc, tc, cfg, dram_in, psums, const_a, acc,
                            zpool, xpool, spool, tpool, bass, mybir,
                            ablate=ablate)

            # --- drain results (ACT is idle at the end; it is also the
            # engine physically closest to PSUM)
            if "noact" not in ablate:
                nc.sync.dma_start(acc_out[:], acc[:])
            drains = []
            if "nope" not in ablate:
                drains += [1, 3]  # z-psums
                if "noscan" not in ablate:
                    drains += [0, 2]  # e-psums
            for i in drains:
                dsb = opool.tile([cfg.blk, 2 * cfg.blk], f32, tag=f"d{i}")
                nc.scalar.activation(
                    dsb[:], psums[i][:], mybir.ActivationFunctionType.Copy
                )
                nc.sync.dma_start(dots_out[i, :, :], dsb[:])

    if loop_n > 1:
        # populate .instr bytes for ISA-encoded instructions inside the
        # For_i block (otherwise codegen fails with "ISA wrong length")
        mybir.codegen_inst_isa_subclasses(nc)
    if split_waits:
        _split_multiwaits(nc)
    return nc


def _split_multiwaits(nc):
    """Engine instructions hold at most ONE sync wait in core_v3 ISA structs
    (walrus: 'Too many sync wait commands'). Tile sometimes attaches 2+.
    Move extras onto same-engine NoOps inserted just before the instruction
    (sequencer executes them in order, so semantics are identical)."""
    import concourse.mybir as mybir

    for f in nc.m.functions:
        for blk in f.blocks:
            out = []
            changed = False
            for ins in blk.instructions:
                si = ins.sync_info
                cap = 2 if isinstance(ins, mybir.InstEventSemaphore) else 1
                if si is not None and si.on_wait and len(si.on_wait) > cap:
                    waits = list(si.on_wait)
                    for w in waits[:-cap]:
                        out.append(
                            mybir.InstNoOp(
                                name=nc.get_next_instruction_name(),
                                engine=ins.engine,
                                ins=[],
                                outs=[],
                                sync_info=mybir.SyncInfo(on_wait=[w], on_update=[]),
                            )
                        )
                    ins.sync_info = mybir.SyncInfo(
                        on_wait=waits[-cap:], on_update=list(si.on_update or [])
                    )
                    changed = True
                out.append(ins)
            if changed:
                blk.instructions = out


def host_combine(results, cfg: Cfg):
    """Combine per-core dots into (start_loss, end_loss, total).

    total_pair = tr(De[:, :B]) + tr(De[:, B:]) - tr(Dz[:, :B]) + tr(Dz[:, B:])
    """
    n_elem = np.float64(B_FULL) * cfg.T
    losses = []
    B = cfg.blk
    for pi in range(2):
        s = np.float64(0.0)
        for res in results:
            dots = np.asarray(res["dots"], dtype=np.float64)
            acc = np.asarray(res["acc"], dtype=np.float64)
            de, dz = dots[2 * pi], dots[2 * pi + 1]
            s += np.trace(de[:, 0:B]) + np.trace(de[:, B : 2 * B])
            s += -np.trace(dz[:, 0:B]) + np.trace(dz[:, B : 2 * B])
            s += 2.0 * acc[:, pi * cfg.n_x : (pi + 1) * cfg.n_x].sum()
        losses.append(s / n_elem)
    start_loss, end_loss = losses
    total = (start_loss + end_loss) / 2.0
    return (
        np.float32(start_loss),
        np.float32(end_loss),
        np.float32(total),
    )


def make_in_maps(inputs, cfg: Cfg):
    """Shard full inputs across cores; host sends zt = 2z-1 padded with -1.
    Both tensors are pre-cast to fp16 (exact for zt; x rounding randomizes
    out in the mean)."""
    H = cfg.halo
    in_maps = []
    for k in range(N_CORES):
        rs = slice(k * ROWS, (k + 1) * ROWS)
        m = {}
        for px, pz in PAIRS:
            m[px] = np.asarray(inputs[px])[rs].astype(np.float16)
            zp = np.full((ROWS, cfg.T + 2 * H), -1.0, dtype=np.float16)
            zp[:, H : H + cfg.T] = (
                2.0 * np.asarray(inputs[pz])[rs] - 1.0
            ).astype(np.float16)
            m[pz] = zp
        in_maps.append(m)
    return in_maps


_NC_CACHE = {}
TRACE = False
LAST_RESULT = None


def kernel(**inputs):
    from concourse.bass_utils import run_bass_kernel_spmd

    cfg = PROD_CFG
    key = "prod"
    if key not in _NC_CACHE:
        _NC_CACHE[key] = build_nc(cfg)
    nc = _NC_CACHE[key]

    in_maps = make_in_maps(inputs, cfg)
    res = run_bass_kernel_spmd(
        nc, in_maps, core_ids=list(range(N_CORES)), trace=TRACE
    )
    global LAST_RESULT
    LAST_RESULT = res
    return host_combine(res.results, cfg)


# revision 46
# speedup vs baseline: 1.1823x; 1.1823x over previous
"""Trainium2 Bass kernel for nn_BoundaryDetectionLoss.

Computes, for start/end (probs, targets) pairs of shape (64, 131072):
    w   = 1 + exp(-dist_to_nearest_boundary / 5)     (distance transform)
    bce = (1-z)*x + (1+z)*softplus(-x)               (pos_weight = 2)
    loss = mean(bce * w)   per pair; total = (start_loss + end_loss)/2

Identity used on device (per pair, z in {0,1}, e = exp(-dist/5), zt = 2z-1):
    sum(bce*w) = sum(x*e) + sum(sp*e) - sum(x*zt) + sum(sp*zt) + 2*sum(sp)
where sp = softplus(-x).  Proof: expanding (1+e)((1-z)x + (1+z)sp) with
z*e = z (e == 1 exactly at boundaries) gives
    sum(x+sp) + sum((x+sp)e) + 2 sum((sp-x)z);
-sum(x*zt) = sum(x) - 2 sum(x*z) folds the plain x sum into the zt dot, and
+sum(sp*zt) = 2 sum(sp*z) - sum(sp) leaves a 2*sum(sp) residue, recovered
for free from the softplus pass's accum_out.

Device algorithm (per core, data-parallel over 8 rows of B=64):
  - Host sends zt = 2*z - 1 (padded with -1) and x, both pre-cast to fp16
    (halves HBM traffic; x rounding errors are random-sign and average out
    ~1e-6 in the mean, z/zt are exact in fp16). Plain HWDGE DMAs.
  - e[t] = exp(-dist[t]/5) as a decayed-max field via two DVE
    tensor_tensor_scan passes (op0=mult by a=exp(-1/5), op1=max) over
    halo'd sub-windows; scan state is fp32 internally.  max(a*state, -1)
    never binds because state >= 0, so the zt-valued scan input produces
    the same field as z would.
  - sp = softplus(-x) = ln(1 + exp(-x)) via two ACT passes per x quarter
    (Exp then Ln with bias=1, both in the natural_log_exp_and_others table
    set -- one table load; Softplus itself is absent from production PWP).
  - PE: per 128-block, psum_e += e_blk^T @ [x|sp], psum_z += zt_blk^T @ [x|sp];
    host sums the PSUM diagonals and combines with the identity above.
"""

import sys

for _p in ("/opt/trn_rl_repo", "/root/.axon_site/_ro/trn_rl_repo"):
    if _p not in sys.path:
        sys.path.append(_p)

import numpy as np

# ---------------------------------------------------------------- config
B_FULL = 64
T_FULL = 131072
N_CORES = 8
ROWS = B_FULL // N_CORES  # 8 rows per core
DECAY = float(np.exp(np.float32(-0.2), dtype=np.float32))  # a = exp(-1/5)


class Cfg:
    def __init__(self, rows=8, chunks=16, scan_splits=(4096,), n_x=4, halo=96):
        self.rows = rows
        self.chunks = chunks
        self.halo = halo
        self.chunk_len = T_FULL // chunks  # 8192
        self.T = T_FULL
        self.parts = rows * chunks
        assert self.parts <= 128
        # scan sub-tiles per chunk: boundaries at scan_splits (asymmetric ok)
        bounds = [0, *scan_splits, self.chunk_len]
        self.subs = [
            (bounds[i], bounds[i + 1] - bounds[i]) for i in range(len(bounds) - 1)
        ]
        self.n_scan = len(self.subs)
        self.max_W = max(ln for _, ln in self.subs) + 2 * halo
        self.n_x = n_x        # x quarters per chunk (ACT granularity)
        self.XL = self.chunk_len // n_x
        self.blk = 128
        self.n_blk = self.chunk_len // 128  # 64
        self.x_ring = "sp"   # which DMA ring carries x: act | sp | gpsimd


PROD_CFG = Cfg()
PAIRS = (("start_probs", "start_targets"), ("end_probs", "end_targets"))


def _build_body(nc, tc, cfg, dram_in, psums, const_a, acc, zpool, xpool,
                spool, tpool, bass, mybir, ablate=frozenset()):
    f32 = mybir.dt.float32
    bf16 = mybir.dt.float16
    AF = mybir.ActivationFunctionType
    OP = mybir.AluOpType
    P, H, CL = cfg.parts, cfg.halo, cfg.chunk_len
    Tp = cfg.T + 2 * H  # host-padded row length

    # Per-pair persistent tiles
    zt = {}   # zt (2z-1) with halo window, bf16
    xs = {}   # [x | sp] stacked moving operand, bf16
    es = {}   # per scan sub-tile: final e field (bwd output), bf16

    def dma_z_seg(pi, i):
        # Disjoint dest segments (seg 0 = first scan window, the rest up to
        # the next window end) so sub-scans start as their data lands
        # without overlapping DMA writes.  SP HWDGE ring.
        px, pz = PAIRS[pi]
        zd = dram_in[pz]
        if pi not in zt:
            zt[pi] = zpool.tile([P, CL + 2 * H], bf16, tag="zt", name=f"zt{pi}")
        ends = [st + ln + 2 * H for st, ln in cfg.subs]
        bounds = [0] + ends
        lo, hi = bounds[i], bounds[i + 1]
        win = bass.AP(
            zd,
            lo,
            [[Tp, cfg.rows], [CL, cfg.chunks], [1, hi - lo]],
        )
        nc.sync.dma_start(zt[pi][:, lo:hi], win)

    def dma_x(pi, h):
        # x halves; SP ring measured fastest (ACT-ring DMAs contend with
        # ACT compute dispatch: 112.6us vs 99.0us; gpsimd/SWDGE: 112.0us).
        px, pz = PAIRS[pi]
        xd = dram_in[px]
        HL = CL // 2
        if pi not in xs:
            xs[pi] = xpool.tile([P, 2 * CL], bf16, tag="xs", name=f"xs{pi}")
        win = bass.AP(
            xd,
            h * HL,
            [[cfg.T, cfg.rows], [CL, cfg.chunks], [1, HL]],
        )
        eng = {"act": nc.scalar, "sp": nc.sync, "gpsimd": nc.gpsimd}[cfg.x_ring]
        eng.dma_start(xs[pi][:, h * HL : (h + 1) * HL], win)

    def scan_fwd(pi, s):
        # ef_s = fwd decayed-max over window [start, start + len + 2H)
        st, ln = cfg.subs[s]
        W = ln + 2 * H
        ef = spool.tile([P, W], bf16, tag="ef", name=f"ef{pi}_{s}")
        zw = zt[pi][:, st : st + W]
        # 1-elem same-engine op absorbs the extra sync waits (STT-class ops
        # have one ISA wait slot; see _split_multiwaits)
        nc.vector.tensor_tensor(ef[:, 0:1], zw[:, 0:1], const_a[:, 0:1], OP.max)
        nc.vector.tensor_tensor_scan(ef[:], const_a[:, :W], zw, 0.0,
                                     OP.mult, OP.max)
        return ef

    def scan_bwd(pi, s, ef):
        st, ln = cfg.subs[s]
        W = ln + 2 * H
        e = spool.tile([P, W], bf16, tag="es", name=f"es{pi}_{s}")
        nc.vector.tensor_tensor(e[:, 0:1], ef[:, 0:1], const_a[:, 0:1], OP.max)
        nc.vector.tensor_tensor_scan(e[:, ::-1], const_a[:, W - 1::-1],
                                     ef[:, ::-1], 0.0, OP.mult, OP.max)
        es[(pi, s)] = e

    def act_sp(pi, q):
        sl = slice(q * cfg.XL, (q + 1) * cfg.XL)
        c = pi * cfg.n_x + q
        texp = tpool.tile([P, cfg.XL], f32, tag="texp", name=f"texp{pi}_{q}")
        nc.scalar.activation(texp[:], xs[pi][:, sl], AF.Exp, scale=-1.0)
        nc.scalar.activation(
            xs[pi][:, CL + q * cfg.XL : CL + (q + 1) * cfg.XL],
            texp[:], AF.Ln, bias=1.0,
            accum_out=acc[:, c : c + 1],
        )

    def pe_blocks(pi, blo, bhi):
        # z-dots first (gated only on zt DMA + sp), then e-dots (gated on
        # the bwd scan, which finishes later) — keeps PE fed while the last
        # scan is still running.
        xs3 = xs[pi][:].rearrange("p (g f) -> p g f", g=2)
        for b in range(blo, bhi):
            z_blk = zt[pi][:, H + b * cfg.blk : H + (b + 1) * cfg.blk]
            rhs = xs3[:, :, b * cfg.blk : (b + 1) * cfg.blk]
            nc.tensor.matmul(psums[2 * pi + 1][:], z_blk, rhs,
                             start=b == 0, stop=b == cfg.n_blk - 1)
        for b in range(blo, bhi):
            pos = b * cfg.blk
            s = max(i for i, (st, ln) in enumerate(cfg.subs) if st <= pos)
            if (pi, s) not in es:
                continue  # scan ablated
            st, ln = cfg.subs[s]
            off = pos - st + H  # offset within es window
            e_blk = es[(pi, s)][:, off : off + cfg.blk]
            rhs = xs3[:, :, pos : pos + cfg.blk]
            nc.tensor.matmul(psums[2 * pi][:], e_blk, rhs,
                             start=b == 0, stop=b == cfg.n_blk - 1)

    # ---- issue order: pipelined across pairs
    # DMA ring (single HWDGE/SP queue, FIFO): first z seg for the scan chain
    # (DVE is the longest engine), then the first x quarter for ACT, then
    # the rest.
    dma_z_seg(0, 0)
    dma_x(0, 0)
    for i in range(1, cfg.n_scan):
        dma_z_seg(0, i)
    dma_x(0, 1)
    for i in range(cfg.n_scan):
        dma_z_seg(1, i)
    for h in range(2):
        dma_x(1, h)

    if "noact" not in ablate:
        for pi in range(2):
            for q in range(cfg.n_x):
                act_sp(pi, q)
    else:
        # sp half of xs must still be written once for the PE reads
        for pi in range(2):
            nc.vector.memset(xs[pi][:, CL : 2 * CL], 0.5)

    if "noscan" not in ablate:
        for pi in range(2):
            for s in range(cfg.n_scan):
                ef = scan_fwd(pi, s)
                scan_bwd(pi, s, ef)

    if "nope" not in ablate:
        # PE per scan sub-tile granularity
        for pi in range(2):
            bper = cfg.n_blk // cfg.n_scan
            for s in range(cfg.n_scan):
                pe_blocks(pi, s * bper, (s + 1) * bper)


def build_nc(cfg: Cfg, split_waits=True, loop_n=1, ablate=frozenset()):
    """Build the per-core Bass program. Returns nc."""
    import concourse.bass as bass
    import concourse.tile as tile
    import concourse.mybir as mybir

    f32 = mybir.dt.float32

    nc = bass.Bass()
    f16 = mybir.dt.float16
    dram_in = {}
    for px, pz in PAIRS:
        dram_in[px] = nc.dram_tensor(px, [cfg.rows, cfg.T], f16, kind="ExternalInput")
        # zt arrives host-transformed (2z-1) and host-padded with -1, halo H
        dram_in[pz] = nc.dram_tensor(
            pz, [cfg.rows, cfg.T + 2 * cfg.halo], f16, kind="ExternalInput"
        )
    # dots: [pair*2+{e,z}, blk, 2*blk]
    dots_out = nc.dram_tensor(
        "dots", [4, cfg.blk, 2 * cfg.blk], f32, kind="ExternalOutput"
    )
    n_acc = 2 * cfg.n_x
    acc_out = nc.dram_tensor("acc", [cfg.parts, n_acc], f32, kind="ExternalOutput")

    with tile.TileContext(nc) as tc:
        with (
            tc.tile_pool(name="const", bufs=1) as cpool,
            tc.tile_pool(name="zt", bufs=2) as zpool,
            tc.tile_pool(name="xs", bufs=2) as xpool,
            tc.tile_pool(name="scan", bufs=2 * cfg.n_scan) as spool,
            tc.tile_pool(name="texp", bufs=2) as tpool,
            tc.tile_pool(name="psum", bufs=1, space="PSUM") as ppool,
            tc.tile_pool(name="outp", bufs=1) as opool,
        ):
            bf16 = mybir.dt.float16
            const_a = cpool.tile([cfg.parts, cfg.max_W], bf16, tag="ca")
            nc.vector.memset(const_a[:], DECAY)
            acc = cpool.tile([cfg.parts, 2 * cfg.n_x], f32, tag="acc")

            psums = [
                ppool.tile([cfg.blk, 2 * cfg.blk], f32, tag=f"ps{i}", name=f"ps{i}")
                for i in range(4)
            ]

            import contextlib

            loop_cm = (
                tc.For_i(0, loop_n, 1, hint_engines=(mybir.EngineType.PE,))
                if loop_n > 1
                else contextlib.nullcontext()
            )
            with loop_cm:
                _build_body(nc, tc, cfg, dram_in, psums, const_a, acc,
                            zpool, xpool, spool, tpool, bass, mybir,
                            ablate=ablate)

            # --- drain results (ACT is idle at the end; it is also the
            # engine physically closest to PSUM)
            if "noact" not in ablate:
                nc.sync.dma_start(acc_out[:], acc[:])
            drains = []
            if "nope" not in ablate:
                drains += [1, 3]  # z-psums
                if "noscan" not in ablate:
                    drains += [0, 2]  # e-psums
            for i in drains:
                dsb = opool.tile([cfg.blk, 2 * cfg.blk], f32, tag=f"d{i}")
                nc.scalar.activation(
                    dsb[:], psums[i][:], mybir.ActivationFunctionType.Copy
                )
                nc.sync.dma_start(dots_out[i, :, :], dsb[:])

    if loop_n > 1:
        # populate .instr bytes for ISA-encoded instructions inside the
        # For_i block (otherwise codegen fails with "ISA wrong length")
        mybir.codegen_inst_isa_subclasses(nc)
    if split_waits:
        _split_multiwaits(nc)
    return nc


def _split_multiwaits(nc):
    """Engine instructions hold at most ONE sync wait in core_v3 ISA structs
    (walrus: 'Too many sync wait commands'). Tile sometimes attaches 2+.
    Move extras onto same-engine NoOps inserted just before the instruction
    (sequencer executes them in order, so semantics are identical)."""
    import concourse.mybir as mybir

    for f in nc.m.functions:
        for blk in f.blocks:
            out = []
            changed = False
            for ins in blk.instructions:
                si = ins.sync_info
                cap = 2 if isinstance(ins, mybir.InstEventSemaphore) else 1
                if si is not None and si.on_wait and len(si.on_wait) > cap:
                    waits = list(si.on_wait)
                    for w in waits[:-cap]:
                        out.append(
                            mybir.InstNoOp(
                                name=nc.get_next_instruction_name(),
                                engine=ins.engine,
                                ins=[],
                                outs=[],
                                sync_info=mybir.SyncInfo(on_wait=[w], on_update=[]),
                            )
                        )
                    ins.sync_info = mybir.SyncInfo(
                        on_wait=waits[-cap:], on_update=list(si.on_update or [])
                    )
                    changed = True
                out.append(ins)
            if changed:
                blk.instructions = out


def host_combine(results, cfg: Cfg):
    """Combine per-core dots into (start_loss, end_loss, total).

    total_pair = tr(De[:, :B]) + tr(De[:, B:]) - tr(Dz[:, :B]) + tr(Dz[:, B:])
    """
    n_elem = np.float64(B_FULL) * cfg.T
    losses = []
    B = cfg.blk
    for pi in range(2):
        s = np.float64(0.0)
        for res in results:
            dots = np.asarray(res["dots"], dtype=np.float64)
            acc = np.asarray(res["acc"], dtype=np.float64)
            de, dz = dots[2 * pi], dots[2 * pi + 1]
            s += np.trace(de[:, 0:B]) + np.trace(de[:, B : 2 * B])
            s += -np.trace(dz[:, 0:B]) + np.trace(dz[:, B : 2 * B])
            s += 2.0 * acc[:, pi * cfg.n_x : (pi + 1) * cfg.n_x].sum()
        losses.append(s / n_elem)
    start_loss, end_loss = losses
    total = (start_loss + end_loss) / 2.0
    return (
        np.float32(start_loss),
        np.float32(end_loss),
        np.float32(total),
    )


def make_in_maps(inputs, cfg: Cfg):
    """Shard full inputs across cores; host sends zt = 2z-1 padded with -1.
    Both tensors are pre-cast to fp16 (exact for zt; x rounding randomizes
    out in the mean)."""
    H = cfg.halo
    in_maps = []
    for k in range(N_CORES):
        rs = slice(k * ROWS, (k + 1) * ROWS)
        m = {}
        for px, pz in PAIRS:
            m[px] = np.asarray(inputs[px])[rs].astype(np.float16)
            zp = np.full((ROWS, cfg.T + 2 * H), -1.0, dtype=np.float16)
            zp[:, H : H + cfg.T] = (
                2.0 * np.asarray(inputs[pz])[rs] - 1.0
            ).astype(np.float16)
            m[pz] = zp
        in_maps.append(m)
    return in_maps


_NC_CACHE = {}
TRACE = False
LAST_RESULT = None


def kernel(**inputs):
    from concourse.bass_utils import run_bass_kernel_spmd

    cfg = PROD_CFG
    key = "prod"
    if key not in _NC_CACHE:
        _NC_CACHE[key] = build_nc(cfg)
    nc = _NC_CACHE[key]

    in_maps = make_in_maps(inputs, cfg)
    res = run_bass_kernel_spmd(
        nc, in_maps, core_ids=list(range(N_CORES)), trace=TRACE
    )
    global LAST_RESULT
    LAST_RESULT = res
    return host_combine(res.results, cfg)


# revision 47
# speedup vs baseline: 1.2284x; 1.0391x over previous
"""Trainium2 Bass kernel for nn_BoundaryDetectionLoss.

Computes, for start/end (probs, targets) pairs of shape (64, 131072):
    w   = 1 + exp(-dist_to_nearest_boundary / 5)     (distance transform)
    bce = (1-z)*x + (1+z)*softplus(-x)               (pos_weight = 2)
    loss = mean(bce * w)   per pair; total = (start_loss + end_loss)/2

Identity used on device (per pair, z in {0,1}, e = exp(-dist/5), zt = 2z-1):
    sum(bce*w) = sum(x*e) + sum(sp*e) - sum(x*zt) + sum(sp*zt) + 2*sum(sp)
where sp = softplus(-x).  Proof: expanding (1+e)((1-z)x + (1+z)sp) with
z*e = z (e == 1 exactly at boundaries) gives
    sum(x+sp) + sum((x+sp)e) + 2 sum((sp-x)z);
-sum(x*zt) = sum(x) - 2 sum(x*z) folds the plain x sum into the zt dot, and
+sum(sp*zt) = 2 sum(sp*z) - sum(sp) leaves a 2*sum(sp) residue, recovered
for free from the softplus pass's accum_out.

Device algorithm (per core, data-parallel over 8 rows of B=64):
  - Host sends zt = 2*z - 1 (padded with -1) and x, both pre-cast to fp16
    (halves HBM traffic; x rounding errors are random-sign and average out
    ~1e-6 in the mean, z/zt are exact in fp16). Plain HWDGE DMAs.
  - e[t] = exp(-dist[t]/5) as a decayed-max field via two DVE
    tensor_tensor_scan passes (op0=mult by a=exp(-1/5), op1=max) over
    halo'd sub-windows; scan state is fp32 internally.  max(a*state, -1)
    never binds because state >= 0, so the zt-valued scan input produces
    the same field as z would.
  - sp = softplus(-x) = ln(1 + exp(-x)) via two ACT passes per x quarter
    (Exp then Ln with bias=1, both in the natural_log_exp_and_others table
    set -- one table load; Softplus itself is absent from production PWP).
  - PE: per 128-block, psum_e += e_blk^T @ [x|sp], psum_z += zt_blk^T @ [x|sp];
    host sums the PSUM diagonals and combines with the identity above.
"""

import sys

for _p in ("/opt/trn_rl_repo", "/root/.axon_site/_ro/trn_rl_repo"):
    if _p not in sys.path:
        sys.path.append(_p)

import numpy as np

# ---------------------------------------------------------------- config
B_FULL = 64
T_FULL = 131072
N_CORES = 8
ROWS = B_FULL // N_CORES  # 8 rows per core
DECAY = float(np.exp(np.float32(-0.2), dtype=np.float32))  # a = exp(-1/5)


class Cfg:
    def __init__(self, rows=8, chunks=16, scan_splits=(4096,), n_x=4, halo=48):
        self.rows = rows
        self.chunks = chunks
        self.halo = halo
        self.chunk_len = T_FULL // chunks  # 8192
        self.T = T_FULL
        self.parts = rows * chunks
        assert self.parts <= 128
        # scan sub-tiles per chunk: boundaries at scan_splits (asymmetric ok)
        bounds = [0, *scan_splits, self.chunk_len]
        self.subs = [
            (bounds[i], bounds[i + 1] - bounds[i]) for i in range(len(bounds) - 1)
        ]
        self.n_scan = len(self.subs)
        self.max_W = max(ln for _, ln in self.subs) + 2 * halo
        self.n_x = n_x        # x quarters per chunk (ACT granularity)
        self.XL = self.chunk_len // n_x
        self.blk = 128
        self.n_blk = self.chunk_len // 128  # 64
        self.x_ring = "sp"   # which DMA ring carries x: act | sp | gpsimd


PROD_CFG = Cfg()
PAIRS = (("start_probs", "start_targets"), ("end_probs", "end_targets"))


def _build_body(nc, tc, cfg, dram_in, psums, const_a, acc, zpool, xpool,
                spool, tpool, bass, mybir, ablate=frozenset()):
    f32 = mybir.dt.float32
    bf16 = mybir.dt.float16
    AF = mybir.ActivationFunctionType
    OP = mybir.AluOpType
    P, H, CL = cfg.parts, cfg.halo, cfg.chunk_len
    Tp = cfg.T + 2 * H  # host-padded row length

    # Per-pair persistent tiles
    zt = {}   # zt (2z-1) with halo window, bf16
    xs = {}   # [x | sp] stacked moving operand, bf16
    es = {}   # per scan sub-tile: final e field (bwd output), bf16

    def dma_z_seg(pi, i):
        # Disjoint dest segments (seg 0 = first scan window, the rest up to
        # the next window end) so sub-scans start as their data lands
        # without overlapping DMA writes.  SP HWDGE ring.
        px, pz = PAIRS[pi]
        zd = dram_in[pz]
        if pi not in zt:
            zt[pi] = zpool.tile([P, CL + 2 * H], bf16, tag="zt", name=f"zt{pi}")
        ends = [st + ln + 2 * H for st, ln in cfg.subs]
        bounds = [0] + ends
        lo, hi = bounds[i], bounds[i + 1]
        win = bass.AP(
            zd,
            lo,
            [[Tp, cfg.rows], [CL, cfg.chunks], [1, hi - lo]],
        )
        nc.sync.dma_start(zt[pi][:, lo:hi], win)

    def dma_x(pi, h):
        # x halves; SP ring measured fastest (ACT-ring DMAs contend with
        # ACT compute dispatch: 112.6us vs 99.0us; gpsimd/SWDGE: 112.0us).
        px, pz = PAIRS[pi]
        xd = dram_in[px]
        HL = CL // 2
        if pi not in xs:
            xs[pi] = xpool.tile([P, 2 * CL], bf16, tag="xs", name=f"xs{pi}")
        win = bass.AP(
            xd,
            h * HL,
            [[cfg.T, cfg.rows], [CL, cfg.chunks], [1, HL]],
        )
        eng = {"act": nc.scalar, "sp": nc.sync, "gpsimd": nc.gpsimd}[cfg.x_ring]
        eng.dma_start(xs[pi][:, h * HL : (h + 1) * HL], win)

    def scan_fwd(pi, s):
        # ef_s = fwd decayed-max over window [start, start + len + 2H)
        st, ln = cfg.subs[s]
        W = ln + 2 * H
        ef = spool.tile([P, W], bf16, tag="ef", name=f"ef{pi}_{s}")
        zw = zt[pi][:, st : st + W]
        # 1-elem same-engine op absorbs the extra sync waits (STT-class ops
        # have one ISA wait slot; see _split_multiwaits)
        nc.vector.tensor_tensor(ef[:, 0:1], zw[:, 0:1], const_a[:, 0:1], OP.max)
        nc.vector.tensor_tensor_scan(ef[:], const_a[:, :W], zw, 0.0,
                                     OP.mult, OP.max)
        return ef

    def scan_bwd(pi, s, ef):
        st, ln = cfg.subs[s]
        W = ln + 2 * H
        e = spool.tile([P, W], bf16, tag="es", name=f"es{pi}_{s}")
        nc.vector.tensor_tensor(e[:, 0:1], ef[:, 0:1], const_a[:, 0:1], OP.max)
        nc.vector.tensor_tensor_scan(e[:, ::-1], const_a[:, W - 1::-1],
                                     ef[:, ::-1], 0.0, OP.mult, OP.max)
        es[(pi, s)] = e

    def act_sp(pi, q):
        sl = slice(q * cfg.XL, (q + 1) * cfg.XL)
        c = pi * cfg.n_x + q
        texp = tpool.tile([P, cfg.XL], f32, tag="texp", name=f"texp{pi}_{q}")
        nc.scalar.activation(texp[:], xs[pi][:, sl], AF.Exp, scale=-1.0)
        nc.scalar.activation(
            xs[pi][:, CL + q * cfg.XL : CL + (q + 1) * cfg.XL],
            texp[:], AF.Ln, bias=1.0,
            accum_out=acc[:, c : c + 1],
        )

    def pe_blocks(pi, blo, bhi):
        # z-dots first (gated only on zt DMA + sp), then e-dots (gated on
        # the bwd scan, which finishes later) — keeps PE fed while the last
        # scan is still running.
        xs3 = xs[pi][:].rearrange("p (g f) -> p g f", g=2)
        for b in range(blo, bhi):
            z_blk = zt[pi][:, H + b * cfg.blk : H + (b + 1) * cfg.blk]
            rhs = xs3[:, :, b * cfg.blk : (b + 1) * cfg.blk]
            nc.tensor.matmul(psums[2 * pi + 1][:], z_blk, rhs,
                             start=b == 0, stop=b == cfg.n_blk - 1)
        for b in range(blo, bhi):
            pos = b * cfg.blk
            s = max(i for i, (st, ln) in enumerate(cfg.subs) if st <= pos)
            if (pi, s) not in es:
                continue  # scan ablated
            st, ln = cfg.subs[s]
            off = pos - st + H  # offset within es window
            e_blk = es[(pi, s)][:, off : off + cfg.blk]
            rhs = xs3[:, :, pos : pos + cfg.blk]
            nc.tensor.matmul(psums[2 * pi][:], e_blk, rhs,
                             start=b == 0, stop=b == cfg.n_blk - 1)

    # ---- issue order: pipelined across pairs
    # DMA ring (single HWDGE/SP queue, FIFO): first z seg for the scan chain
    # (DVE is the longest engine), then the first x quarter for ACT, then
    # the rest.
    dma_z_seg(0, 0)
    dma_x(0, 0)
    for i in range(1, cfg.n_scan):
        dma_z_seg(0, i)
    dma_x(0, 1)
    for i in range(cfg.n_scan):
        dma_z_seg(1, i)
    for h in range(2):
        dma_x(1, h)

    if "noact" not in ablate:
        for pi in range(2):
            for q in range(cfg.n_x):
                act_sp(pi, q)
    else:
        # sp half of xs must still be written once for the PE reads
        for pi in range(2):
            nc.vector.memset(xs[pi][:, CL : 2 * CL], 0.5)

    if "noscan" not in ablate:
        for pi in range(2):
            for s in range(cfg.n_scan):
                ef = scan_fwd(pi, s)
                scan_bwd(pi, s, ef)

    if "nope" not in ablate:
        # PE per scan sub-tile granularity
        for pi in range(2):
            bper = cfg.n_blk // cfg.n_scan
            for s in range(cfg.n_scan):
                pe_blocks(pi, s * bper, (s + 1) * bper)


def build_nc(cfg: Cfg, split_waits=True, loop_n=1, ablate=frozenset()):
    """Build the per-core Bass program. Returns nc."""
    import concourse.bass as bass
    import concourse.tile as tile
    import concourse.mybir as mybir

    f32 = mybir.dt.float32

    nc = bass.Bass()
    f16 = mybir.dt.float16
    dram_in = {}
    for px, pz in PAIRS:
        dram_in[px] = nc.dram_tensor(px, [cfg.rows, cfg.T], f16, kind="ExternalInput")
        # zt arrives host-transformed (2z-1) and host-padded with -1, halo H
        dram_in[pz] = nc.dram_tensor(
            pz, [cfg.rows, cfg.T + 2 * cfg.halo], f16, kind="ExternalInput"
        )
    # dots: [pair*2+{e,z}, blk, 2*blk]
    dots_out = nc.dram_tensor(
        "dots", [4, cfg.blk, 2 * cfg.blk], f32, kind="ExternalOutput"
    )
    n_acc = 2 * cfg.n_x
    acc_out = nc.dram_tensor("acc", [cfg.parts, n_acc], f32, kind="ExternalOutput")

    with tile.TileContext(nc) as tc:
        with (
            tc.tile_pool(name="const", bufs=1) as cpool,
            tc.tile_pool(name="zt", bufs=2) as zpool,
            tc.tile_pool(name="xs", bufs=2) as xpool,
            tc.tile_pool(name="scan", bufs=2 * cfg.n_scan) as spool,
            tc.tile_pool(name="texp", bufs=2) as tpool,
            tc.tile_pool(name="psum", bufs=1, space="PSUM") as ppool,
            tc.tile_pool(name="outp", bufs=1) as opool,
        ):
            bf16 = mybir.dt.float16
            const_a = cpool.tile([cfg.parts, cfg.max_W], bf16, tag="ca")
            nc.vector.memset(const_a[:], DECAY)
            acc = cpool.tile([cfg.parts, 2 * cfg.n_x], f32, tag="acc")

            psums = [
                ppool.tile([cfg.blk, 2 * cfg.blk], f32, tag=f"ps{i}", name=f"ps{i}")
                for i in range(4)
            ]

            import contextlib

            loop_cm = (
                tc.For_i(0, loop_n, 1, hint_engines=(mybir.EngineType.PE,))
                if loop_n > 1
                else contextlib.nullcontext()
            )
            with loop_cm:
                _build_body(nc, tc, cfg, dram_in, psums, const_a, acc,
                            zpool, xpool, spool, tpool, bass, mybir,
                            ablate=ablate)

            # --- drain results (ACT is idle at the end; it is also the
            # engine physically closest to PSUM)
            if "noact" not in ablate:
                nc.sync.dma_start(acc_out[:], acc[:])
            drains = []
            if "nope" not in ablate:
                drains += [1, 3]  # z-psums
                if "noscan" not in ablate:
                    drains += [0, 2]  # e-psums
            for i in drains:
                dsb = opool.tile([cfg.blk, 2 * cfg.blk], f32, tag=f"d{i}")
                nc.scalar.activation(
                    dsb[:], psums[i][:], mybir.ActivationFunctionType.Copy
                )
                nc.sync.dma_start(dots_out[i, :, :], dsb[:])

    if loop_n > 1:
        # populate .instr bytes for ISA-encoded instructions inside the
        # For_i block (otherwise codegen fails with "ISA wrong length")
        mybir.codegen_inst_isa_subclasses(nc)
    if split_waits:
        _split_multiwaits(nc)
    return nc


def _split_multiwaits(nc):
    """Engine instructions hold at most ONE sync wait in core_v3 ISA structs
    (walrus: 'Too many sync wait commands'). Tile sometimes attaches 2+.
    Move extras onto same-engine NoOps inserted just before the instruction
    (sequencer executes them in order, so semantics are identical)."""
    import concourse.mybir as mybir

    for f in nc.m.functions:
        for blk in f.blocks:
            out = []
            changed = False
            for ins in blk.instructions:
                si = ins.sync_info
                cap = 2 if isinstance(ins, mybir.InstEventSemaphore) else 1
                if si is not None and si.on_wait and len(si.on_wait) > cap:
                    waits = list(si.on_wait)
                    for w in waits[:-cap]:
                        out.append(
                            mybir.InstNoOp(
                                name=nc.get_next_instruction_name(),
                                engine=ins.engine,
                                ins=[],
                                outs=[],
                                sync_info=mybir.SyncInfo(on_wait=[w], on_update=[]),
                            )
                        )
                    ins.sync_info = mybir.SyncInfo(
                        on_wait=waits[-cap:], on_update=list(si.on_update or [])
                    )
                    changed = True
                out.append(ins)
            if changed:
                blk.instructions = out


def host_combine(results, cfg: Cfg):
    """Combine per-core dots into (start_loss, end_loss, total).

    total_pair = tr(De[:, :B]) + tr(De[:, B:]) - tr(Dz[:, :B]) + tr(Dz[:, B:])
    """
    n_elem = np.float64(B_FULL) * cfg.T
    losses = []
    B = cfg.blk
    for pi in range(2):
        s = np.float64(0.0)
        for res in results:
            dots = np.asarray(res["dots"], dtype=np.float64)
            acc = np.asarray(res["acc"], dtype=np.float64)
            de, dz = dots[2 * pi], dots[2 * pi + 1]
            s += np.trace(de[:, 0:B]) + np.trace(de[:, B : 2 * B])
            s += -np.trace(dz[:, 0:B]) + np.trace(dz[:, B : 2 * B])
            s += 2.0 * acc[:, pi * cfg.n_x : (pi + 1) * cfg.n_x].sum()
        losses.append(s / n_elem)
    start_loss, end_loss = losses
    total = (start_loss + end_loss) / 2.0
    return (
        np.float32(start_loss),
        np.float32(end_loss),
        np.float32(total),
    )


def make_in_maps(inputs, cfg: Cfg):
    """Shard full inputs across cores; host sends zt = 2z-1 padded with -1.
    Both tensors are pre-cast to fp16 (exact for zt; x rounding randomizes
    out in the mean)."""
    H = cfg.halo
    in_maps = []
    for k in range(N_CORES):
        rs = slice(k * ROWS, (k + 1) * ROWS)
        m = {}
        for px, pz in PAIRS:
            m[px] = np.asarray(inputs[px])[rs].astype(np.float16)
            zp = np.full((ROWS, cfg.T + 2 * H), -1.0, dtype=np.float16)
            zp[:, H : H + cfg.T] = (
                2.0 * np.asarray(inputs[pz])[rs] - 1.0
            ).astype(np.float16)
            m[pz] = zp
        in_maps.append(m)
    return in_maps


_NC_CACHE = {}
TRACE = False
LAST_RESULT = None


def kernel(**inputs):
    from concourse.bass_utils import run_bass_kernel_spmd

    cfg = PROD_CFG
    key = "prod"
    if key not in _NC_CACHE:
        _NC_CACHE[key] = build_nc(cfg)
    nc = _NC_CACHE[key]

    in_maps = make_in_maps(inputs, cfg)
    res = run_bass_kernel_spmd(
        nc, in_maps, core_ids=list(range(N_CORES)), trace=TRACE
    )
    global LAST_RESULT
    LAST_RESULT = res
    return host_combine(res.results, cfg)
